# revision 41
# baseline (speedup 1.0000x reference)
"""Multi-head GQA attention prefill (B=1, S=2048, D=4096, 32 q-heads /
8 kv-heads, head_dim=128, RoPE, causal) on 8 TRN2 NeuronCores.

Sharding: tensor-parallel over heads (core c owns q-heads [4c,4c+4) and
kv-head c); wo sharded over rows (output columns) with an AllGather of
the normalized attention outputs before the out-projection.

Schedule (v2, PE-density-first):
  phase P: all projections (one pass, 6 PSUM banks: pq0-3, pk, pvT),
           x streamed once; RoPE + V-copy ride on DVE/ScalarE only --
           no PE ops between proj blocks, so the PE stream is gapless.
           V is produced pre-transposed by extra N=128 matmuls
           (pvT[s,hd] = xt_chunk.T @ wv_chunk) -- no PE transposes.
  phase A: attention ascending sb (0..3); flat (h,kc) chunk stream with
           the PV/rowsum matmuls lagging the QK matmuls by 2 chunks so
           ScalarE exp latency never stalls the PE.  Softmax denominators
           accumulate via a ones-column matmul; normalization
           (reciprocal -> bf16 broadcast matmul -> DVE mul) and the
           AllGather are injected into the chunk stream as fillers.
  phase O: out-projection (4x4 blocks of 32 accumulating matmuls),
           acol gathers prefetched 3 blocks ahead.  All AGs are hidden
           under attention/outproj compute.

RoPE trick: scores are invariant under a fixed permutation of head dims
applied to both q and k, so wq/wk rows are host-deinterleaved to
[evens; odds].  RoPE is then dst = psum*[cos;cos] + swap(psum)*[-sin;sin]
(ScalarE does the half-swap, DVE does 3 full-width ops).
"""

import sys

sys.path.insert(0, "/opt/trn_rl_repo")

from collections import deque

import numpy as np
import ml_dtypes

import concourse.bass as bass
import concourse.mybir as mybir
from concourse import bacc, tile
from concourse.bass_utils import run_bass_kernel_spmd

F32 = mybir.dt.float32
BF16 = mybir.dt.bfloat16
BF16_NP = ml_dtypes.bfloat16

NCORES = 8
S = 2048
D = 4096
HD = 128                 # head dim
QH = 4                   # q heads per core
QROWS = QH * HD          # 512 q rows per core
SB = 512                 # seq superblock
NSB = S // SB            # 4
DC = D // 128            # 32 contraction chunks
NKC = S // 128           # 16 key chunks
SCALE = 1.0 / np.sqrt(HD)


def build_graph():
    nc = bacc.Bacc("TRN2", target_bir_lowering=False, debug=False,
                   num_devices=NCORES)

    xT = nc.declare_dram_parameter("xT", [D, S], BF16, isOutput=False)
    # weights pre-arranged on host to [128, DC, cols] so every DMA is
    # contiguous per partition (the (c p)->p c rearrange pattern generates
    # ~1k descriptors per transfer and starves the projection phase).
    wqT = nc.declare_dram_parameter("wqT", [128, DC, QROWS], BF16,
                                    isOutput=False)
    wkT = nc.declare_dram_parameter("wkT", [128, DC, HD], BF16,
                                    isOutput=False)
    wvT = nc.declare_dram_parameter("wvT", [128, DC, HD], BF16,
                                    isOutput=False)
    woT = nc.declare_dram_parameter("woT", [128, DC, SB], BF16,
                                    isOutput=False)
    cos2 = nc.declare_dram_parameter("cos2", [128, S], F32, isOutput=False)
    sin2 = nc.declare_dram_parameter("sin2", [128, S], F32, isOutput=False)
    mask = nc.declare_dram_parameter("mask", [128, 128], BF16, isOutput=False)
    out = nc.declare_dram_parameter("out", [S, SB], F32, isOutput=True)

    aT_loc = [nc.dram_tensor(f"aT_loc{sb}", [QROWS, SB], BF16)
              for sb in range(NSB)]
    aT_all = [nc.dram_tensor(f"aT_all{sb}", [NCORES * QROWS, SB], BF16,
                             addr_space="Shared") for sb in range(NSB)]

    with tile.TileContext(nc) as tc:
        with tc.tile_pool(name="const", bufs=1) as cpool, \
             tc.tile_pool(name="wts", bufs=1) as wpool, \
             tc.tile_pool(name="qkv", bufs=1) as qkvpool, \
             tc.tile_pool(name="xs", bufs=16) as xpool, \
             tc.tile_pool(name="rope", bufs=2) as rpool, \
             tc.tile_pool(name="exps", bufs=4) as epool, \
             tc.tile_pool(name="onorm", bufs=2) as opool, \
             tc.tile_pool(name="ostream", bufs=4) as spool, \
             tc.tile_pool(name="ps", bufs=8, space="PSUM") as ps:

            # ---- weights: q first (proj pass order is pq0..3, pk, pvT) ----
            wq_sb = wpool.tile([128, DC, QROWS], BF16, tag="wq")
            wk_sb = wpool.tile([128, DC, HD], BF16, tag="wk")
            wv_sb = wpool.tile([128, DC, HD], BF16, tag="wv")
            wo_sb = wpool.tile([128, DC, SB], BF16, tag="wo")
            G = 8
            # geometric ramp: tiny first chunks so the first matmuls can
            # start ~5us in instead of waiting on a 1MB transfer.
            bounds = [0, 1, 2, 4, 8, 16, 24, 32]
            for a, b in zip(bounds, bounds[1:]):
                s = slice(a, b)
                nc.gpsimd.dma_start(wq_sb[:, s, :], wqT[:, s, :])
                nc.scalar.dma_start(wk_sb[:, s, :], wkT[:, s, :])
                nc.scalar.dma_start(wv_sb[:, s, :], wvT[:, s, :])

            # ---- constants (cos/sin after the early weight groups so the
            # startup DMA burst doesn't starve proj(0)) ----
            mask_t = cpool.tile([128, 128], BF16, tag="mask")
            nc.scalar.dma_start(mask_t[:], mask[:])
            ones_col = cpool.tile([128, 1], BF16, tag="ones_col")
            nc.vector.memset(ones_col[:], 1.0)
            ones_all = cpool.tile([128, 128], BF16, tag="ones_all")
            nc.vector.memset(ones_all[:], 1.0)
            warm_t = cpool.tile([1, 8], BF16, tag="warm")
            # pre-warm the ScalarE Exp table off the critical path.
            nc.scalar.activation(warm_t[:], ones_all[0:1, 0:8],
                                 mybir.ActivationFunctionType.Exp, scale=1.0)
            cos_t = cpool.tile([128, S], F32, tag="cos")
            nc.scalar.dma_start(cos_t[:], cos2[:, :])
            sin_t = cpool.tile([128, S], F32, tag="sin")
            nc.scalar.dma_start(sin_t[:], sin2[:, :])

            def make_wo_dma(g):
                def f():
                    nc.gpsimd.dma_start(wo_sb[:, g:g + G, :],
                                        woT[:, g:g + G, :])
                return f

            # ---- persistent activations ----
            qT = [qkvpool.tile([128, S], BF16, tag=f"qT{h}", name=f"qT{h}")
                  for h in range(QH)]
            kT = qkvpool.tile([128, S], BF16, tag="kT")
            v_sb = [qkvpool.tile([128, HD], BF16, tag=f"v{kc}", name=f"v{kc}")
                    for kc in range(NKC)]

            # ================= phase P: projections =================
            # rolling x prefetch: dc-groups issued in global order across
            # sb boundaries, always ~2 groups ahead of consumption.
            xts = {}
            xq = [(sbq, g) for sbq in range(NSB) for g in range(DC // G)]
            xq_pos = [0]

            def issue_xgroup():
                if xq_pos[0] >= len(xq):
                    return
                sbq, g = xq[xq_pos[0]]
                xq_pos[0] += 1
                cols = bass.ts(sbq, SB)
                for dc in range(g * G, (g + 1) * G):
                    xt = xpool.tile([128, SB], BF16, tag="xt", bufs=24,
                                    name=f"xt{sbq}_{dc}")
                    nc.sync.dma_start(xt[:], xT[bass.ts(dc, 128), cols])
                    xts[(sbq, dc)] = xt

            def emit_proj(sb):
                pq = [ps.tile([128, SB], F32, tag="ps", name=f"pq{sb}_{h}")
                      for h in range(QH)]
                pk = ps.tile([128, SB], F32, tag="ps", name=f"pk{sb}")
                pv = ps.tile([128, SB], F32, tag="ps", name=f"pv{sb}")
                if sb == 0:
                    issue_xgroup()
                    issue_xgroup()
                for g in range(DC // G):
                    issue_xgroup()
                    rng = range(g * G, (g + 1) * G)
                    for h in range(QH):
                        for dc in rng:
                            nc.tensor.matmul(pq[h][:],
                                             wq_sb[:, dc, bass.ts(h, HD)],
                                             xts[(sb, dc)][:],
                                             start=dc == 0, stop=dc == DC - 1)
                    for dc in rng:
                        nc.tensor.matmul(pk[:], wk_sb[:, dc, :],
                                         xts[(sb, dc)][:],
                                         start=dc == 0, stop=dc == DC - 1)
                    for dc in rng:
                        nc.tensor.matmul(pv[:], wv_sb[:, dc, :],
                                         xts[(sb, dc)][:],
                                         start=dc == 0, stop=dc == DC - 1)
                    for dc in rng:
                        del xts[(sb, dc)]
                return pq, pk, pv

            def rope(psrc, dst, cols):
                t1 = rpool.tile([128, SB], F32, tag="t1")
                nc.vector.tensor_mul(t1[:], psrc[:], cos_t[:, cols])
                t2 = rpool.tile([128, SB], F32, tag="t2")
                nc.vector.tensor_mul(t2[0:64, :], psrc[64:128, :],
                                     sin_t[0:64, cols])
                nc.vector.tensor_mul(t2[64:128, :], psrc[0:64, :],
                                     sin_t[64:128, cols])
                nc.vector.tensor_add(dst[:, cols], t1[:], t2[:])

            def emit_rope_v(sb, tiles):
                pq, pk, pv = tiles
                cols = bass.ts(sb, SB)
                vt = rpool.tile([128, SB], BF16, tag="vt")
                nc.vector.tensor_copy(vt[:], pv[:])
                for j in range(4):
                    nc.sync.dma_start_transpose(v_sb[4 * sb + j][:],
                                                vt[:, bass.ts(j, 128)])
                for h in range(QH):
                    rope(pq[h], qT[h], cols)
                rope(pk, kT, cols)

            # ============ filler plumbing (norm / AG / acol) ============
            fills = deque()

            def fill():
                if fills:
                    fills.popleft()()

            norm_state = {}

            def norm_head_steps(sb, h, po, rps):
                """Per-head: stash rowsums into sm4 rows {0,32,64,96} (frees
                the rps bank) and copy po out to bf16 (frees the po bank)."""
                def s_sm():
                    if h == 0:
                        sm4 = opool.tile([128, SB], F32, tag="sm4", bufs=2,
                                         name=f"sm4_{sb}")
                        nc.vector.memset(sm4[:], 1.0)
                        norm_state[sb] = {"sm4": sm4}
                    sm4 = norm_state[sb]["sm4"]
                    nc.vector.tensor_copy(sm4[32 * h:32 * h + 1, :], rps[:])

                def s_au():
                    au = opool.tile([128, SB], BF16, tag="au", bufs=5,
                                    name=f"au{sb}_{h}")
                    nc.vector.tensor_copy(au[:], po[:])
                    norm_state[sb][h] = au

                return [s_sm, s_au]

            def norm_tail_steps(sb):
                """After all 4 heads: one batched reciprocal + cast, then
                staggered pb broadcasts, normalize muls, stores, AllGather."""
                st = {}

                def s_recip_a():
                    sm4 = norm_state[sb]["sm4"]
                    rcf4 = opool.tile([128, SB], F32, tag="rcf4", bufs=2,
                                      name=f"rcf4_{sb}")
                    nc.vector.reciprocal(rcf4[:, 0:SB // 2],
                                         sm4[:, 0:SB // 2])
                    st["rcf4"] = rcf4

                def s_recip_b():
                    sm4 = norm_state[sb]["sm4"]
                    rcf4 = st["rcf4"]
                    nc.vector.reciprocal(rcf4[:, SB // 2:SB],
                                         sm4[:, SB // 2:SB])
                    rcb4 = opool.tile([128, SB], BF16, tag="rcb4", bufs=2,
                                      name=f"rcb4_{sb}")
                    nc.vector.tensor_copy(rcb4[:], rcf4[:])
                    st["rcb4"] = rcb4
                    # partition base 96 is not a legal matmul base; stage
                    # head 3's row down at partition 0.
                    rcbh3 = opool.tile([1, SB], BF16, tag="rcbh3", bufs=2,
                                       name=f"rcbh3_{sb}")
                    nc.vector.tensor_copy(rcbh3[:], rcb4[96:97, :])
                    st["rcbh3"] = rcbh3

                def make_pb(hpair):
                    def s_pb():
                        rcb4 = st["rcb4"]
                        for h in hpair:
                            pb = ps.tile([128, SB], F32, tag="ps",
                                         name=f"pb{sb}_{h}")
                            if h == 3:
                                lhs, rhs = ones_all[0:1, :], st["rcbh3"][:]
                            else:
                                lhs = ones_all[32 * h:32 * h + 1, :]
                                rhs = rcb4[32 * h:32 * h + 1, :]
                            nc.tensor.matmul(pb[:], lhs, rhs,
                                             start=True, stop=True)
                            st[h] = pb
                    return s_pb

                def make_at(hpair, last):
                    def s_at():
                        for h in hpair:
                            pb = st.pop(h)
                            au = norm_state[sb].pop(h)
                            at = opool.tile([128, SB], BF16, tag="at", bufs=2,
                                            name=f"at{sb}_{h}")
                            nc.vector.tensor_mul(at[:], au[:], pb[:])
                            nc.sync.dma_start(aT_loc[sb][bass.ts(h, 128), :],
                                              at[:])
                        if last:
                            nc.gpsimd.collective_compute(
                                "AllGather",
                                mybir.AluOpType.bypass,
                                ins=[aT_loc[sb][:]],
                                outs=[aT_all[sb][:]],
                                replica_groups=[list(range(NCORES))],
                            )
                    return s_at

                return [s_recip_a, s_recip_b, lambda: None, lambda: None,
                        make_pb((0, 1)), make_at((0, 1), False),
                        make_pb((2, 3)), make_at((2, 3), True)]

            # ================= phase A: attention =================
            def emit_attn(sb):
                nkc = 4 * sb + 4
                chunks = [(h, kc) for h in range(QH) for kc in range(nkc)]
                state = {}
                es = {}

                def qk(h, kc):
                    j = kc - 4 * sb
                    c0 = 128 * j if j > 0 else 0
                    pg = ps.tile([128, SB], F32, tag="ps",
                                 name=f"pg{sb}_{h}_{kc}")
                    nc.tensor.matmul(
                        pg[:, c0:SB], kT[:, bass.ts(kc, 128)],
                        qT[h][:, sb * SB + c0:(sb + 1) * SB],
                        start=True, stop=True)
                    e = epool.tile([128, SB], BF16, tag="es",
                                   name=f"es{sb}_{h}_{kc}")
                    nc.scalar.activation(e[:, c0:SB], pg[:, c0:SB],
                                         mybir.ActivationFunctionType.Exp,
                                         scale=SCALE)
                    if j >= 0:
                        nc.vector.tensor_mul(e[:, bass.ts(j, 128)],
                                             e[:, bass.ts(j, 128)], mask_t[:])
                    es[(h, kc)] = e

                def pv(h, kc):
                    po, rps = state[h]
                    j = kc - 4 * sb
                    c0 = 128 * j if j > 0 else 0
                    st, sp = kc == 0, kc == nkc - 1
                    e = es.pop((h, kc))
                    nc.tensor.matmul(po[:, c0:SB], v_sb[kc][:], e[:, c0:SB],
                                     start=st, stop=sp)
                    nc.tensor.matmul(rps[:, c0:SB], ones_col[:], e[:, c0:SB],
                                     start=st, stop=sp)
                    if sp:
                        fills.extend(norm_head_steps(sb, h, po, rps))
                        if h == QH - 1:
                            fills.extend(norm_tail_steps(sb))

                for i, (h, kc) in enumerate(chunks):
                    if kc == 0:
                        po = ps.tile([128, SB], F32, tag="ps",
                                     name=f"po{sb}_{h}")
                        rps = ps.tile([1, SB], F32, tag="ps",
                                      name=f"rps{sb}_{h}")
                        state[h] = (po, rps)
                    qk(h, kc)
                    fill()
                    if i >= 2:
                        pv(*chunks[i - 2])
                pv(*chunks[-2])
                pv(*chunks[-1])

            # ================= phase O: out-projection =================
            # attention & outproj process sb in order [1,2,3,0]: sb=1's first
            # chunks carry no diagonal-mask DVE dependency, so they overlap
            # the rope(3) DVE backlog at the proj->attn boundary (sb=0 is
            # all-diagonal and would stall ~6us there).
            SB_ORDER = [1, 2, 3, 0]
            OJ = [(sbp, j) for sbp in SB_ORDER for j in range(4)]
            acols = {}

            def make_acol_issue(i):
                def f():
                    sbp, j = OJ[i]
                    acol = spool.tile([128, DC, 128], BF16, tag="acol",
                                      bufs=3, name=f"acol{sbp}_{j}")
                    nc.sync.dma_start(
                        acol[:],
                        aT_all[sbp][:, bass.ts(j, 128)].rearrange(
                            "(c p) m -> p c m", p=128))
                    acols[(sbp, j)] = acol
                return f

            def emit_outproj(i):
                sbp, j = OJ[i]
                mc = 4 * sbp + j
                while (sbp, j) not in acols and fills:
                    fill()
                acol = acols.pop((sbp, j))
                pout = ps.tile([128, SB], F32, tag="ps", name=f"pout{sbp}_{j}")
                if i + 3 < len(OJ):
                    fills.append(make_acol_issue(i + 3))
                for hc in range(DC):
                    nc.tensor.matmul(pout[:], acol[:, hc, :], wo_sb[:, hc, :],
                                     start=hc == 0, stop=hc == DC - 1)
                    if hc % 4 == 3:
                        fill()
                ot = spool.tile([128, SB], F32, tag="ot", bufs=2,
                                name=f"ot{sbp}_{j}")
                nc.vector.tensor_copy(ot[:], pout[:])
                nc.sync.dma_start(out[bass.ts(mc, 128), :], ot[:])

            # ---- schedule ----
            for sb in range(NSB):
                tiles = emit_proj(sb)
                emit_rope_v(sb, tiles)
            for oi, sb in enumerate(SB_ORDER):
                if oi == 0:
                    # wo weights deferred off the startup DMA burst; pull
                    # them in on the idle gpsimd ring during attention.
                    for g in range(0, DC, G):
                        fills.append(make_wo_dma(g))
                if oi == len(SB_ORDER) - 1:
                    for i in range(3):
                        fills.append(make_acol_issue(i))
                emit_attn(sb)
            for i in range(len(OJ)):
                emit_outproj(i)
            while fills:
                fills.popleft()()

    nc.compile()
    return nc


_PERM = np.concatenate([np.arange(0, HD, 2), np.arange(1, HD, 2)])


def _prep_inputs(x, wq, wk, wv, wo, freqs_cos, freqs_sin):
    xT = np.ascontiguousarray(x.reshape(S, D).T).astype(BF16_NP)
    cosT = np.ascontiguousarray(freqs_cos.T).astype(np.float32)   # [64, S]
    sinT = np.ascontiguousarray(freqs_sin.T).astype(np.float32)
    cos2 = np.ascontiguousarray(np.concatenate([cosT, cosT], axis=0))
    sin2 = np.ascontiguousarray(np.concatenate([-sinT, sinT], axis=0))
    mask = np.triu(np.ones((128, 128), dtype=np.float32)).astype(BF16_NP)

    qperm = np.concatenate([h * HD + _PERM for h in range(QH)])

    def pcm(wT):
        # [D, cols] -> [128, DC, cols] so per-partition DMA reads are
        # contiguous.
        cols = wT.shape[1]
        return np.ascontiguousarray(
            wT.reshape(DC, 128, cols).swapaxes(0, 1)).astype(BF16_NP)

    in_maps = []
    for c in range(NCORES):
        wq_c = wq[c * QROWS:(c + 1) * QROWS][qperm]
        wk_c = wk[c * HD:(c + 1) * HD][_PERM]
        wv_c = wv[c * HD:(c + 1) * HD]
        wo_c = wo[c * SB:(c + 1) * SB]
        in_maps.append({
            "xT": xT,
            "wqT": pcm(wq_c.T.astype(np.float32)),
            "wkT": pcm(wk_c.T.astype(np.float32)),
            "wvT": pcm(wv_c.T.astype(np.float32)),
            "woT": pcm(wo_c.T.astype(np.float32)),
            "cos2": cos2,
            "sin2": sin2,
            "mask": mask,
        })
    return in_maps


def kernel(x, wq, wk, wv, wo, freqs_cos, freqs_sin, start_pos=0, *,
           _trace=False):
    x = np.asarray(x, dtype=np.float32)
    in_maps = _prep_inputs(np.asarray(x, np.float32), np.asarray(wq, np.float32),
                           np.asarray(wk, np.float32), np.asarray(wv, np.float32),
                           np.asarray(wo, np.float32),
                           np.asarray(freqs_cos, np.float32),
                           np.asarray(freqs_sin, np.float32))
    nc = build_graph()
    res = run_bass_kernel_spmd(nc, in_maps, core_ids=list(range(NCORES)),
                               trace=_trace)
    full = np.concatenate([res.results[c]["out"] for c in range(NCORES)],
                          axis=1)
    out = full.reshape(1, S, D).astype(np.float32)
    if _trace:
        return out, res
    return out


# revision 43
# speedup vs baseline: 1.0007x; 1.0007x over previous
"""Multi-head GQA attention prefill (B=1, S=2048, D=4096, 32 q-heads /
8 kv-heads, head_dim=128, RoPE, causal) on 8 TRN2 NeuronCores.

Sharding: tensor-parallel over heads (core c owns q-heads [4c,4c+4) and
kv-head c); wo sharded over rows (output columns) with an AllGather of
the normalized attention outputs before the out-projection.

Schedule (v2, PE-density-first):
  phase P: all projections (one pass, 6 PSUM banks: pq0-3, pk, pvT),
           x streamed once; RoPE + V-copy ride on DVE/ScalarE only --
           no PE ops between proj blocks, so the PE stream is gapless.
           V is produced pre-transposed by extra N=128 matmuls
           (pvT[s,hd] = xt_chunk.T @ wv_chunk) -- no PE transposes.
  phase A: attention ascending sb (0..3); flat (h,kc) chunk stream with
           the PV/rowsum matmuls lagging the QK matmuls by 2 chunks so
           ScalarE exp latency never stalls the PE.  Softmax denominators
           accumulate via a ones-column matmul; normalization
           (reciprocal -> bf16 broadcast matmul -> DVE mul) and the
           AllGather are injected into the chunk stream as fillers.
  phase O: out-projection (4x4 blocks of 32 accumulating matmuls),
           acol gathers prefetched 3 blocks ahead.  All AGs are hidden
           under attention/outproj compute.

RoPE trick: scores are invariant under a fixed permutation of head dims
applied to both q and k, so wq/wk rows are host-deinterleaved to
[evens; odds].  RoPE is then dst = psum*[cos;cos] + swap(psum)*[-sin;sin]
(ScalarE does the half-swap, DVE does 3 full-width ops).
"""

import sys

sys.path.insert(0, "/opt/trn_rl_repo")

from collections import deque

import numpy as np
import ml_dtypes

import concourse.bass as bass
import concourse.mybir as mybir
from concourse import bacc, tile
from concourse.bass_utils import run_bass_kernel_spmd

F32 = mybir.dt.float32
BF16 = mybir.dt.bfloat16
BF16_NP = ml_dtypes.bfloat16

NCORES = 8
S = 2048
D = 4096
HD = 128                 # head dim
QH = 4                   # q heads per core
QROWS = QH * HD          # 512 q rows per core
SB = 512                 # seq superblock
NSB = S // SB            # 4
DC = D // 128            # 32 contraction chunks
NKC = S // 128           # 16 key chunks
SCALE = 1.0 / np.sqrt(HD)


def build_graph():
    nc = bacc.Bacc("TRN2", target_bir_lowering=False, debug=False,
                   num_devices=NCORES)

    xT = nc.declare_dram_parameter("xT", [D, S], BF16, isOutput=False)
    # weights pre-arranged on host to [128, DC, cols] so every DMA is
    # contiguous per partition (the (c p)->p c rearrange pattern generates
    # ~1k descriptors per transfer and starves the projection phase).
    wqT = nc.declare_dram_parameter("wqT", [128, DC, QROWS], BF16,
                                    isOutput=False)
    wkT = nc.declare_dram_parameter("wkT", [128, DC, HD], BF16,
                                    isOutput=False)
    wvT = nc.declare_dram_parameter("wvT", [128, DC, HD], BF16,
                                    isOutput=False)
    woT = nc.declare_dram_parameter("woT", [128, DC, SB], BF16,
                                    isOutput=False)
    cos2 = nc.declare_dram_parameter("cos2", [128, S], F32, isOutput=False)
    sin2 = nc.declare_dram_parameter("sin2", [128, S], F32, isOutput=False)
    mask = nc.declare_dram_parameter("mask", [128, 128], BF16, isOutput=False)
    out = nc.declare_dram_parameter("out", [S, SB], F32, isOutput=True)

    aT_loc = [nc.dram_tensor(f"aT_loc{sb}", [QROWS, SB], BF16)
              for sb in range(NSB)]
    aT_all = [nc.dram_tensor(f"aT_all{sb}", [NCORES * QROWS, SB], BF16,
                             addr_space="Shared") for sb in range(NSB)]

    with tile.TileContext(nc) as tc:
        with tc.tile_pool(name="const", bufs=1) as cpool, \
             tc.tile_pool(name="wts", bufs=1) as wpool, \
             tc.tile_pool(name="qkv", bufs=1) as qkvpool, \
             tc.tile_pool(name="xs", bufs=16) as xpool, \
             tc.tile_pool(name="rope", bufs=2) as rpool, \
             tc.tile_pool(name="exps", bufs=6) as epool, \
             tc.tile_pool(name="onorm", bufs=2) as opool, \
             tc.tile_pool(name="ostream", bufs=4) as spool, \
             tc.tile_pool(name="ps", bufs=8, space="PSUM") as ps:

            # ---- weights: q first (proj pass order is pq0..3, pk, pvT) ----
            wq_sb = wpool.tile([128, DC, QROWS], BF16, tag="wq")
            wk_sb = wpool.tile([128, DC, HD], BF16, tag="wk")
            wv_sb = wpool.tile([128, DC, HD], BF16, tag="wv")
            wo_sb = wpool.tile([128, DC, SB], BF16, tag="wo")
            G = 8
            # geometric ramp: tiny first chunks so the first matmuls can
            # start ~5us in instead of waiting on a 1MB transfer.
            bounds = [0, 1, 2, 4, 8, 16, 24, 32]
            for a, b in zip(bounds, bounds[1:]):
                s = slice(a, b)
                nc.gpsimd.dma_start(wq_sb[:, s, :], wqT[:, s, :])
                nc.scalar.dma_start(wk_sb[:, s, :], wkT[:, s, :])
                nc.scalar.dma_start(wv_sb[:, s, :], wvT[:, s, :])

            # ---- constants (cos/sin after the early weight groups so the
            # startup DMA burst doesn't starve proj(0)) ----
            mask_t = cpool.tile([128, 128], BF16, tag="mask")
            nc.scalar.dma_start(mask_t[:], mask[:])
            ones_col = cpool.tile([128, 1], BF16, tag="ones_col")
            nc.vector.memset(ones_col[:], 1.0)
            ones_all = cpool.tile([128, 128], BF16, tag="ones_all")
            nc.vector.memset(ones_all[:], 1.0)
            warm_t = cpool.tile([1, 8], BF16, tag="warm")
            # pre-warm the ScalarE Exp table off the critical path.
            nc.scalar.activation(warm_t[:], ones_all[0:1, 0:8],
                                 mybir.ActivationFunctionType.Exp, scale=1.0)
            cos_t = cpool.tile([128, S], F32, tag="cos")
            nc.scalar.dma_start(cos_t[:], cos2[:, :])
            sin_t = cpool.tile([128, S], F32, tag="sin")
            nc.scalar.dma_start(sin_t[:], sin2[:, :])

            def make_wo_dma(g):
                def f():
                    nc.gpsimd.dma_start(wo_sb[:, g:g + G, :],
                                        woT[:, g:g + G, :])
                return f

            # ---- persistent activations ----
            qT = [qkvpool.tile([128, S], BF16, tag=f"qT{h}", name=f"qT{h}")
                  for h in range(QH)]
            kT = qkvpool.tile([128, S], BF16, tag="kT")
            v_sb = [qkvpool.tile([128, HD], BF16, tag=f"v{kc}", name=f"v{kc}")
                    for kc in range(NKC)]

            # ================= phase P: projections =================
            # rolling x prefetch: dc-groups issued in global order across
            # sb boundaries, always ~2 groups ahead of consumption.
            xts = {}
            xq = [(sbq, g) for sbq in range(NSB) for g in range(DC // G)]
            xq_pos = [0]

            def issue_xgroup():
                if xq_pos[0] >= len(xq):
                    return
                sbq, g = xq[xq_pos[0]]
                xq_pos[0] += 1
                cols = bass.ts(sbq, SB)
                for dc in range(g * G, (g + 1) * G):
                    xt = xpool.tile([128, SB], BF16, tag="xt", bufs=22,
                                    name=f"xt{sbq}_{dc}")
                    nc.sync.dma_start(xt[:], xT[bass.ts(dc, 128), cols])
                    xts[(sbq, dc)] = xt

            def emit_proj(sb):
                pq = [ps.tile([128, SB], F32, tag="ps", name=f"pq{sb}_{h}")
                      for h in range(QH)]
                pk = ps.tile([128, SB], F32, tag="ps", name=f"pk{sb}")
                pv = ps.tile([128, SB], F32, tag="ps", name=f"pv{sb}")
                if sb == 0:
                    issue_xgroup()
                    issue_xgroup()
                for g in range(DC // G):
                    issue_xgroup()
                    rng = range(g * G, (g + 1) * G)
                    for h in range(QH):
                        for dc in rng:
                            nc.tensor.matmul(pq[h][:],
                                             wq_sb[:, dc, bass.ts(h, HD)],
                                             xts[(sb, dc)][:],
                                             start=dc == 0, stop=dc == DC - 1)
                    for dc in rng:
                        nc.tensor.matmul(pk[:], wk_sb[:, dc, :],
                                         xts[(sb, dc)][:],
                                         start=dc == 0, stop=dc == DC - 1)
                    for dc in rng:
                        nc.tensor.matmul(pv[:], wv_sb[:, dc, :],
                                         xts[(sb, dc)][:],
                                         start=dc == 0, stop=dc == DC - 1)
                    for dc in rng:
                        del xts[(sb, dc)]
                return pq, pk, pv

            def rope(psrc, dst, cols):
                t1 = rpool.tile([128, SB], F32, tag="t1")
                nc.vector.tensor_mul(t1[:], psrc[:], cos_t[:, cols])
                t2 = rpool.tile([128, SB], F32, tag="t2")
                nc.vector.tensor_mul(t2[0:64, :], psrc[64:128, :],
                                     sin_t[0:64, cols])
                nc.vector.tensor_mul(t2[64:128, :], psrc[0:64, :],
                                     sin_t[64:128, cols])
                nc.vector.tensor_add(dst[:, cols], t1[:], t2[:])

            def emit_rope_v(sb, tiles):
                pq, pk, pv = tiles
                cols = bass.ts(sb, SB)
                vt = rpool.tile([128, SB], BF16, tag="vt")
                nc.vector.tensor_copy(vt[:], pv[:])
                for j in range(4):
                    nc.sync.dma_start_transpose(v_sb[4 * sb + j][:],
                                                vt[:, bass.ts(j, 128)])
                for h in range(QH):
                    rope(pq[h], qT[h], cols)
                rope(pk, kT, cols)

            # ============ filler plumbing (norm / AG / acol) ============
            fills = deque()

            def fill():
                if fills:
                    fills.popleft()()

            norm_state = {}

            def norm_head_steps(sb, h, po, rps):
                """Per-head: stash rowsums into sm4 rows {0,32,64,96} (frees
                the rps bank) and copy po out to bf16 (frees the po bank)."""
                def s_sm():
                    if h == 0:
                        sm4 = opool.tile([128, SB], F32, tag="sm4", bufs=2,
                                         name=f"sm4_{sb}")
                        nc.vector.memset(sm4[:], 1.0)
                        norm_state[sb] = {"sm4": sm4}
                    sm4 = norm_state[sb]["sm4"]
                    nc.vector.tensor_copy(sm4[32 * h:32 * h + 1, :], rps[:])

                def s_au():
                    au = opool.tile([128, SB], BF16, tag="au", bufs=5,
                                    name=f"au{sb}_{h}")
                    nc.vector.tensor_copy(au[:], po[:])
                    norm_state[sb][h] = au

                return [s_sm, s_au]

            def norm_tail_steps(sb):
                """After all 4 heads: one batched reciprocal + cast, then
                staggered pb broadcasts, normalize muls, stores, AllGather."""
                st = {}

                def s_recip_a():
                    sm4 = norm_state[sb]["sm4"]
                    rcf4 = opool.tile([128, SB], F32, tag="rcf4", bufs=2,
                                      name=f"rcf4_{sb}")
                    nc.vector.reciprocal(rcf4[:, 0:SB // 2],
                                         sm4[:, 0:SB // 2])
                    st["rcf4"] = rcf4

                def s_recip_b():
                    sm4 = norm_state[sb]["sm4"]
                    rcf4 = st["rcf4"]
                    nc.vector.reciprocal(rcf4[:, SB // 2:SB],
                                         sm4[:, SB // 2:SB])
                    rcb4 = opool.tile([128, SB], BF16, tag="rcb4", bufs=2,
                                      name=f"rcb4_{sb}")
                    nc.vector.tensor_copy(rcb4[:], rcf4[:])
                    st["rcb4"] = rcb4
                    # partition base 96 is not a legal matmul base; stage
                    # head 3's row down at partition 0.
                    rcbh3 = opool.tile([1, SB], BF16, tag="rcbh3", bufs=2,
                                       name=f"rcbh3_{sb}")
                    nc.vector.tensor_copy(rcbh3[:], rcb4[96:97, :])
                    st["rcbh3"] = rcbh3

                def make_pb(hpair):
                    def s_pb():
                        rcb4 = st["rcb4"]
                        for h in hpair:
                            pb = ps.tile([128, SB], F32, tag="ps",
                                         name=f"pb{sb}_{h}")
                            if h == 3:
                                lhs, rhs = ones_all[0:1, :], st["rcbh3"][:]
                            else:
                                lhs = ones_all[32 * h:32 * h + 1, :]
                                rhs = rcb4[32 * h:32 * h + 1, :]
                            nc.tensor.matmul(pb[:], lhs, rhs,
                                             start=True, stop=True)
                            st[h] = pb
                    return s_pb

                def make_at(hpair, last):
                    def s_at():
                        for h in hpair:
                            pb = st.pop(h)
                            au = norm_state[sb].pop(h)
                            at = opool.tile([128, SB], BF16, tag="at", bufs=2,
                                            name=f"at{sb}_{h}")
                            nc.vector.tensor_mul(at[:], au[:], pb[:])
                            nc.sync.dma_start(aT_loc[sb][bass.ts(h, 128), :],
                                              at[:])
                        if last:
                            nc.gpsimd.collective_compute(
                                "AllGather",
                                mybir.AluOpType.bypass,
                                ins=[aT_loc[sb][:]],
                                outs=[aT_all[sb][:]],
                                replica_groups=[list(range(NCORES))],
                            )
                    return s_at

                return [s_recip_a, s_recip_b, lambda: None, lambda: None,
                        make_pb((0, 1)), make_at((0, 1), False),
                        make_pb((2, 3)), make_at((2, 3), True)]

            # ================= phase A: attention =================
            def emit_attn(sb):
                nkc = 4 * sb + 4
                chunks = [(h, kc) for h in range(QH) for kc in range(nkc)]
                state = {}
                es = {}

                def qk(h, kc):
                    j = kc - 4 * sb
                    c0 = 128 * j if j > 0 else 0
                    pg = ps.tile([128, SB], F32, tag="ps",
                                 name=f"pg{sb}_{h}_{kc}")
                    nc.tensor.matmul(
                        pg[:, c0:SB], kT[:, bass.ts(kc, 128)],
                        qT[h][:, sb * SB + c0:(sb + 1) * SB],
                        start=True, stop=True)
                    e = epool.tile([128, SB], BF16, tag="es",
                                   name=f"es{sb}_{h}_{kc}")
                    nc.scalar.activation(e[:, c0:SB], pg[:, c0:SB],
                                         mybir.ActivationFunctionType.Exp,
                                         scale=SCALE)
                    if j >= 0:
                        nc.vector.tensor_mul(e[:, bass.ts(j, 128)],
                                             e[:, bass.ts(j, 128)], mask_t[:])
                    es[(h, kc)] = e

                def pv(h, kc):
                    po, rps = state[h]
                    j = kc - 4 * sb
                    c0 = 128 * j if j > 0 else 0
                    st, sp = kc == 0, kc == nkc - 1
                    e = es.pop((h, kc))
                    nc.tensor.matmul(po[:, c0:SB], v_sb[kc][:], e[:, c0:SB],
                                     start=st, stop=sp)
                    nc.tensor.matmul(rps[:, c0:SB], ones_col[:], e[:, c0:SB],
                                     start=st, stop=sp)
                    if sp:
                        fills.extend(norm_head_steps(sb, h, po, rps))
                        if h == QH - 1:
                            fills.extend(norm_tail_steps(sb))

                for i, (h, kc) in enumerate(chunks):
                    if kc == 0:
                        po = ps.tile([128, SB], F32, tag="ps",
                                     name=f"po{sb}_{h}")
                        rps = ps.tile([1, SB], F32, tag="ps",
                                      name=f"rps{sb}_{h}")
                        state[h] = (po, rps)
                    qk(h, kc)
                    fill()
                    if i >= 2:
                        pv(*chunks[i - 2])
                pv(*chunks[-2])
                pv(*chunks[-1])

            # ================= phase O: out-projection =================
            # attention & outproj process sb in order [1,2,3,0]: sb=1's first
            # chunks carry no diagonal-mask DVE dependency, so they overlap
            # the rope(3) DVE backlog at the proj->attn boundary (sb=0 is
            # all-diagonal and would stall ~6us there).
            SB_ORDER = [1, 2, 3, 0]
            OJ = [(sbp, j) for sbp in SB_ORDER for j in range(4)]
            acols = {}

            def make_acol_issue(i):
                def f():
                    sbp, j = OJ[i]
                    acol = spool.tile([128, DC, 128], BF16, tag="acol",
                                      bufs=3, name=f"acol{sbp}_{j}")
                    nc.sync.dma_start(
                        acol[:],
                        aT_all[sbp][:, bass.ts(j, 128)].rearrange(
                            "(c p) m -> p c m", p=128))
                    acols[(sbp, j)] = acol
                return f

            def emit_outproj(i):
                sbp, j = OJ[i]
                mc = 4 * sbp + j
                while (sbp, j) not in acols and fills:
                    fill()
                acol = acols.pop((sbp, j))
                pout = ps.tile([128, SB], F32, tag="ps", name=f"pout{sbp}_{j}")
                if i + 3 < len(OJ):
                    fills.append(make_acol_issue(i + 3))
                for hc in range(DC):
                    nc.tensor.matmul(pout[:], acol[:, hc, :], wo_sb[:, hc, :],
                                     start=hc == 0, stop=hc == DC - 1)
                    if hc % 4 == 3:
                        fill()
                ot = spool.tile([128, SB], F32, tag="ot", bufs=2,
                                name=f"ot{sbp}_{j}")
                nc.vector.tensor_copy(ot[:], pout[:])
                nc.sync.dma_start(out[bass.ts(mc, 128), :], ot[:])

            # ---- schedule ----
            for sb in range(NSB):
                tiles = emit_proj(sb)
                emit_rope_v(sb, tiles)
            for oi, sb in enumerate(SB_ORDER):
                if oi == 0:
                    # wo weights deferred off the startup DMA burst; pull
                    # them in on the idle gpsimd ring during attention.
                    for g in range(0, DC, G):
                        fills.append(make_wo_dma(g))
                if oi == len(SB_ORDER) - 1:
                    for i in range(3):
                        fills.append(make_acol_issue(i))
                emit_attn(sb)
            for i in range(len(OJ)):
                emit_outproj(i)
            while fills:
                fills.popleft()()

    nc.compile()
    return nc


_PERM = np.concatenate([np.arange(0, HD, 2), np.arange(1, HD, 2)])


def _prep_inputs(x, wq, wk, wv, wo, freqs_cos, freqs_sin):
    xT = np.ascontiguousarray(x.reshape(S, D).T).astype(BF16_NP)
    cosT = np.ascontiguousarray(freqs_cos.T).astype(np.float32)   # [64, S]
    sinT = np.ascontiguousarray(freqs_sin.T).astype(np.float32)
    cos2 = np.ascontiguousarray(np.concatenate([cosT, cosT], axis=0))
    sin2 = np.ascontiguousarray(np.concatenate([-sinT, sinT], axis=0))
    mask = np.triu(np.ones((128, 128), dtype=np.float32)).astype(BF16_NP)

    qperm = np.concatenate([h * HD + _PERM for h in range(QH)])

    def pcm(wT):
        # [D, cols] -> [128, DC, cols] so per-partition DMA reads are
        # contiguous.
        cols = wT.shape[1]
        return np.ascontiguousarray(
            wT.reshape(DC, 128, cols).swapaxes(0, 1)).astype(BF16_NP)

    in_maps = []
    for c in range(NCORES):
        wq_c = wq[c * QROWS:(c + 1) * QROWS][qperm]
        wk_c = wk[c * HD:(c + 1) * HD][_PERM]
        wv_c = wv[c * HD:(c + 1) * HD]
        wo_c = wo[c * SB:(c + 1) * SB]
        in_maps.append({
            "xT": xT,
            "wqT": pcm(wq_c.T.astype(np.float32)),
            "wkT": pcm(wk_c.T.astype(np.float32)),
            "wvT": pcm(wv_c.T.astype(np.float32)),
            "woT": pcm(wo_c.T.astype(np.float32)),
            "cos2": cos2,
            "sin2": sin2,
            "mask": mask,
        })
    return in_maps


def kernel(x, wq, wk, wv, wo, freqs_cos, freqs_sin, start_pos=0, *,
           _trace=False):
    x = np.asarray(x, dtype=np.float32)
    in_maps = _prep_inputs(np.asarray(x, np.float32), np.asarray(wq, np.float32),
                           np.asarray(wk, np.float32), np.asarray(wv, np.float32),
                           np.asarray(wo, np.float32),
                           np.asarray(freqs_cos, np.float32),
                           np.asarray(freqs_sin, np.float32))
    nc = build_graph()
    res = run_bass_kernel_spmd(nc, in_maps, core_ids=list(range(NCORES)),
                               trace=_trace)
    full = np.concatenate([res.results[c]["out"] for c in range(NCORES)],
                          axis=1)
    out = full.reshape(1, S, D).astype(np.float32)
    if _trace:
        return out, res
    return out
